# revision 16
# baseline (speedup 1.0000x reference)
"""CenterLoss (segment_reduce) Trainium2 kernel — eq-mask + TensorE matmul.

Math (faithful to the reference):
  preds = argmax_c logits[n, c, h, w]          (softmax is monotone -> skip it)
  per (n, cls): cnt = #pixels with preds==cls,
                S1 = sum over those pixels of sum_c x,
                S2 = sum over those pixels of sum_c x^2
  K = max(cnt,1)*C; sq_dev = max(S2 - S1^2/K, 0)
  loss = sum_cls mean_n( cnt>0 ? sqrt(sq_dev) : 0 )

Key reformulation: with eq[c, p] = (x[c, p] == max_k x[k, p]) the per-class
sums expand over channels,
  S1[c] = sum_k G[c, k],   G[c, k]  = sum_p eq[c, p] x[k, p]
  S2[c] = sum_k G2[c, k],  G2[c, k] = sum_p eq[c, p] x[k, p]^2
  cnt[c] = sum_p eq[c, p]
so after the masks exist, ALL pixel accumulation is a matmul contracting
over pixels — which the (otherwise idle) TensorEngine does into PSUM.

Device strategy (8 cores, data-parallel over 16 units = (n, H-slab of 128)):
  Everything is bf16 (host casts; halves HBM traffic; validated rel err
  ~3.8e-3 vs the 2e-2 gate — bf16 max ties double-count ~0.5% of pixels).
  Per chunk (128 h-partitions x C x 512 w):
    DVE   : pairwise max tree over C (bf16 2x_1P), then ONE broadcast
            is_equal(x, m) producing all 19 masks (~10.3 us/chunk).
    ACT   : Square(x) into the moving tile's rows 19:38 (~8.4 us/chunk).
    PE    : 128 matmuls (stationary = eq slice (128, 19x4w), moving =
            [x; x^2; ones] slice (128, 39x4w)), PSUM-accumulated so the
            w-sum happens in PSUM; one (76, 156) readout per chunk.
    out   : scalar-engine PSUM->SBUF copy, DMA to HBM (190 KB/core total).
  Host: decode the 4x4 w-phase diagonal, apply the final formula in f64.
"""

import numpy as np


def _ensure_ntff_hook():
    """bass_utils' trace path imports antenv.axon_hooks, which this image
    lacks.  Install a shim backed by trn_agent_boot's ctypes hook so a
    BASS_TRACE=1 environment doesn't crash the run (and tracing works)."""
    import sys
    import types

    try:
        import antenv.axon_hooks  # noqa: F401
        return
    except ImportError:
        pass
    try:
        from trn_agent_boot.trn_boot import _ntff_profile_via_ctypes

        hook = _ntff_profile_via_ctypes("/opt/axon/libaxon_pjrt.so")
    except Exception:
        hook = None
    mod = types.ModuleType("antenv.axon_hooks")
    mod.get_axon_ntff_profile_hook = lambda: hook
    mod.set_axon_ntff_profile_hook = lambda h: None
    sys.modules["antenv.axon_hooks"] = mod


N, C, H, W = 4, 19, 512, 1024
NCORES = 8
SLABS = 4                 # H split into 4 slabs of 128 partitions
P = H // SLABS            # 128
UNITS = [(n, s) for n in range(N) for s in range(SLABS)]   # 16 units
UPC = len(UNITS) // NCORES                                  # 2 units per core
R = 2 * C + 1             # moving rows: [x (19); x^2 (19); ones (1)]
WB = 4                    # w columns batched per matmul
SF = WB * C               # stationary free = 76
MF = WB * R               # moving free = 156
WMAX = 256

# W split per unit: small chunks at the stream head (fast pipeline fill)
# and tail (short drain), 256 in the middle.
WSPLITS = {0: [128, 128, 256, 256, 256], 1: [256, 256, 256, 128, 128]}
# chunk list: (unit, elem offset in the flat per-core buffer, width)
CHUNKS = []
_off = 0
for _u in range(UPC):
    for _wid in WSPLITS[_u]:
        CHUNKS.append((_u, _off, _wid))
        _off += P * C * _wid
TOTELEMS = _off
NSLOTS = len(CHUNKS)

_CACHE = {}


def _build_nc():
    from contextlib import ExitStack

    import concourse.tile as tile
    from concourse import bacc, mybir

    f32 = mybir.dt.float32
    bf16 = mybir.dt.bfloat16
    Alu = mybir.AluOpType
    Act = mybir.ActivationFunctionType

    nc = bacc.Bacc("TRN2", target_bir_lowering=False, debug=False)
    # Host packs each core's shard as consecutive per-chunk-contiguous
    # (h, c, w) blocks so every chunk load is 128 contiguous >=4.8 KB runs.
    x_d = nc.dram_tensor("x", [TOTELEMS], bf16, kind="ExternalInput").ap()
    out_d = nc.dram_tensor(
        "stats", [NSLOTS, SF, MF], f32, kind="ExternalOutput"
    ).ap()

    with tile.TileContext(nc) as tc, ExitStack() as ctx:
        zpool = ctx.enter_context(tc.tile_pool(name="z", bufs=5))
        epool = ctx.enter_context(tc.tile_pool(name="eq", bufs=4))
        tpool = ctx.enter_context(tc.tile_pool(name="tree", bufs=2))
        mpool = ctx.enter_context(tc.tile_pool(name="m", bufs=2))
        pspool = ctx.enter_context(tc.tile_pool(name="ps", bufs=4, space="PSUM"))
        opool = ctx.enter_context(tc.tile_pool(name="o", bufs=3))

        tt = nc.vector.tensor_tensor
        for slot, (_u, off, wid) in enumerate(CHUNKS):
            nmm = wid // WB
            Z = zpool.tile([P, R, WMAX], bf16, tag="z", name=f"z{slot}")
            src = x_d[off:off + P * C * wid].rearrange(
                "(p c w) -> p c w", p=P, c=C, w=wid
            )
            nc.sync.dma_start(Z[:, 0:C, 0:wid], src)
            nc.gpsimd.memset(Z[:, 2 * C, 0:wid], 1.0)
            # squares on ScalarE (otherwise idle), into the moving tile
            nc.scalar.activation(
                Z[:, C:2 * C, 0:wid], Z[:, 0:C, 0:wid], Act.Square
            )

            # pairwise max tree over the 19 channel rows, all bf16 2x
            t = tpool.tile([P, 10, WMAX], bf16, tag="t", name=f"t{slot}")
            x = Z[:, 0:C, 0:wid]
            tv = t[:, :, 0:wid]
            tt(out=tv[:, 0:9], in0=x[:, 0:9], in1=x[:, 9:18], op=Alu.max)
            tt(out=tv[:, 0:4], in0=tv[:, 0:4], in1=tv[:, 4:8], op=Alu.max)
            tt(out=tv[:, 0:2], in0=tv[:, 0:2], in1=tv[:, 2:4], op=Alu.max)
            tt(out=tv[:, 0], in0=tv[:, 0], in1=tv[:, 1], op=Alu.max)
            tt(out=tv[:, 0], in0=tv[:, 0], in1=tv[:, 8], op=Alu.max)
            m = mpool.tile([P, WMAX], bf16, tag="m", name=f"m{slot}")
            mv = m[:, 0:wid]
            tt(out=mv, in0=tv[:, 0], in1=x[:, 18], op=Alu.max)

            # all 19 masks via broadcast compare (bf16 2x_1P), written in
            # w-blocked (g, c, wb) order so each group's stationary slice
            # coalesces to ONE free dim (the PE weights AP requires that);
            # split so the matmul stream starts before the far pieces finish
            eq = epool.tile([P, WMAX // WB, C, WB], bf16, tag="eq",
                            name=f"eq{slot}")
            x_blk = x.rearrange("p c (g b) -> p g c b", b=WB)
            m_blk = mv.rearrange("p (g b) -> p g b", b=WB)
            m_b = m_blk[:, :, None, :].broadcast_to([P, nmm, C, WB])
            nsplit = max(1, nmm // 32)
            qg = nmm // nsplit
            for q in range(nsplit):
                sl = slice(q * qg, (q + 1) * qg)
                tt(out=eq[:, sl], in0=x_blk[:, sl], in1=m_b[:, sl],
                   op=Alu.is_equal)

            # PSUM-accumulated masked sums: out[(c,wi),(r,wj)] +=
            #   sum_p eq[p,c,4g+wi] * Z[p,r,4g+wj]; host keeps wi==wj.
            # pad the PSUM tile to a full 2 KB bank so pool packing never
            # co-locates two accumulators in one bank (false serialization)
            ps_full = pspool.tile([SF, 512], f32, tag="ps", name=f"ps{slot}")
            ps = ps_full[:, 0:MF]
            for g in range(nmm):
                w0 = g * WB
                nc.tensor.matmul(
                    ps,
                    lhsT=eq[:, g],
                    rhs=Z[:, :, w0:w0 + WB],
                    start=(g == 0),
                    stop=(g == nmm - 1),
                )

            ob = opool.tile([SF, MF], f32, tag="o", name=f"o{slot}")
            nc.scalar.copy(out=ob[:], in_=ps)
            nc.sync.dma_start(out_d[slot], ob[:])

    nc.compile()
    return nc


def _get_nc():
    if "nc" not in _CACHE:
        _CACHE["nc"] = _build_nc()
    return _CACHE["nc"]


def _make_shards(logits):
    import ml_dtypes

    xb = np.ascontiguousarray(logits).astype(ml_dtypes.bfloat16)
    shards = []
    for k in range(NCORES):
        flat = np.empty(TOTELEMS, dtype=ml_dtypes.bfloat16)
        for (u, off, wid) in CHUNKS:
            n, s = UNITS[UPC * k + u]
            w0 = sum(w for (uu, _o, w) in CHUNKS if uu == u and _o < off)
            blk = xb[n, :, s * P:(s + 1) * P, w0:w0 + wid]   # (C, P, wid)
            flat[off:off + P * C * wid] = (
                blk.transpose(1, 0, 2).reshape(-1)           # (P, C, wid)
            )
        shards.append(flat)
    return shards


def _finish(results):
    per_n = np.zeros((N, 3, C), dtype=np.float64)
    for k in range(NCORES):
        arr = np.asarray(results[k]["stats"], dtype=np.float64)
        # (NSLOTS, SF=C*WB, MF=R*WB) -> diagonal over the WB w-phases
        a = arr.reshape(NSLOTS, C, WB, R, WB)
        g_all = np.einsum('scirj->scr', a * np.eye(WB)[None, None, :, None, :])
        for slot in range(NSLOTS):
            u, _off, _wid = CHUNKS[slot]
            n, _s = UNITS[UPC * k + u]
            g = g_all[slot]                          # (C, R)
            per_n[n, 0] += g[:, 2 * C]               # cnt
            per_n[n, 1] += g[:, 0:C].sum(axis=1)     # S1
            per_n[n, 2] += g[:, C:2 * C].sum(axis=1)  # S2
    cnt, S1, S2 = per_n[:, 0], per_n[:, 1], per_n[:, 2]
    K = np.maximum(cnt, 1.0) * C
    sq_dev = np.maximum(S2 - S1 * S1 / K, 0.0)
    norms = np.where(cnt > 0, np.sqrt(sq_dev), 0.0)
    loss = norms.mean(axis=0).sum()
    return np.array(loss, dtype=np.float32)


def kernel(**inputs):
    _ensure_ntff_hook()
    from concourse.bass_utils import run_bass_kernel_spmd

    logits = np.asarray(inputs["logits"])
    assert logits.shape == (N, C, H, W), logits.shape
    nc = _get_nc()
    shards = _make_shards(np.asarray(logits, dtype=np.float32))
    in_maps = [{"x": shards[k]} for k in range(NCORES)]
    res = run_bass_kernel_spmd(nc, in_maps, list(range(NCORES)))
    return _finish(res.results)


# revision 19
# speedup vs baseline: 1.1876x; 1.1876x over previous
"""CenterLoss (segment_reduce) Trainium2 kernel — eq-mask + TensorE matmul.

Math (faithful to the reference):
  preds = argmax_c logits[n, c, h, w]          (softmax is monotone -> skip it)
  per (n, cls): cnt = #pixels with preds==cls,
                S1 = sum over those pixels of sum_c x,
                S2 = sum over those pixels of sum_c x^2
  K = max(cnt,1)*C; sq_dev = max(S2 - S1^2/K, 0)
  loss = sum_cls mean_n( cnt>0 ? sqrt(sq_dev) : 0 )

Key reformulation: with eq[c, p] = (x[c, p] == max_k x[k, p]) the per-class
sums expand over channels,
  S1[c] = sum_k G[c, k],   G[c, k]  = sum_p eq[c, p] x[k, p]
  S2[c] = sum_k G2[c, k],  G2[c, k] = sum_p eq[c, p] x[k, p]^2
  cnt[c] = sum_p eq[c, p]
so after the masks exist, ALL pixel accumulation is a matmul contracting
over pixels — which the (otherwise idle) TensorEngine does into PSUM.

Device strategy (8 cores, data-parallel over 16 units = (n, H-slab of 128)):
  Everything is bf16 (host casts; halves HBM traffic; validated rel err
  ~3.8e-3 vs the 2e-2 gate — bf16 max ties double-count ~0.5% of pixels).
  Per chunk (128 h-partitions x C x 512 w):
    DVE   : pairwise max tree over C (bf16 2x_1P), then ONE broadcast
            is_equal(x, m) producing all 19 masks (~10.3 us/chunk).
    ACT   : Square(x) into the moving tile's rows 19:38 (~8.4 us/chunk).
    PE    : 128 matmuls (stationary = eq slice (128, 19x4w), moving =
            [x; x^2; ones] slice (128, 39x4w)), PSUM-accumulated so the
            w-sum happens in PSUM; one (76, 156) readout per chunk.
    out   : scalar-engine PSUM->SBUF copy, DMA to HBM (190 KB/core total).
  Host: decode the 4x4 w-phase diagonal, apply the final formula in f64.
"""

import numpy as np


def _ensure_ntff_hook():
    """bass_utils' trace path imports antenv.axon_hooks, which this image
    lacks.  Install a shim backed by trn_agent_boot's ctypes hook so a
    BASS_TRACE=1 environment doesn't crash the run (and tracing works)."""
    import sys
    import types

    try:
        import antenv.axon_hooks  # noqa: F401
        return
    except ImportError:
        pass
    try:
        from trn_agent_boot.trn_boot import _ntff_profile_via_ctypes

        hook = _ntff_profile_via_ctypes("/opt/axon/libaxon_pjrt.so")
    except Exception:
        hook = None
    mod = types.ModuleType("antenv.axon_hooks")
    mod.get_axon_ntff_profile_hook = lambda: hook
    mod.set_axon_ntff_profile_hook = lambda h: None
    sys.modules["antenv.axon_hooks"] = mod


N, C, H, W = 4, 19, 512, 1024
NCORES = 8
SLABS = 4                 # H split into 4 slabs of 128 partitions
P = H // SLABS            # 128
UNITS = [(n, s) for n in range(N) for s in range(SLABS)]   # 16 units
UPC = len(UNITS) // NCORES                                  # 2 units per core
R = 2 * C + 1             # moving rows: [x (19); x^2 (19); ones (1)]
WB = 4                    # w columns batched per matmul
SF = WB * C               # stationary free = 76
MF = WB * R               # moving free = 156
WMAX = 256

# uniform 256-wide chunks (measured fastest pipeline granularity)
WSPLITS = {u: [256, 256, 256, 256] for u in range(UPC)}
# chunk list: (unit, elem offset in the flat per-core buffer, width)
CHUNKS = []
_off = 0
for _u in range(UPC):
    for _wid in WSPLITS[_u]:
        CHUNKS.append((_u, _off, _wid))
        _off += P * C * _wid
TOTELEMS = _off
NSLOTS = len(CHUNKS)

_CACHE = {}


def _build_nc():
    from contextlib import ExitStack

    import concourse.tile as tile
    from concourse import bacc, mybir

    f32 = mybir.dt.float32
    bf16 = mybir.dt.bfloat16
    Alu = mybir.AluOpType
    Act = mybir.ActivationFunctionType

    nc = bacc.Bacc("TRN2", target_bir_lowering=False, debug=False)
    # Host packs each core's shard as consecutive per-chunk-contiguous
    # (h, c, w) blocks so every chunk load is 128 contiguous >=4.8 KB runs.
    x_d = nc.dram_tensor("x", [TOTELEMS], bf16, kind="ExternalInput").ap()
    out_d = nc.dram_tensor(
        "stats", [NSLOTS, SF, MF], f32, kind="ExternalOutput"
    ).ap()

    with tile.TileContext(nc) as tc, ExitStack() as ctx:
        zpool = ctx.enter_context(tc.tile_pool(name="z", bufs=4))
        epool = ctx.enter_context(tc.tile_pool(name="eq", bufs=4))
        tpool = ctx.enter_context(tc.tile_pool(name="tree", bufs=2))
        mpool = ctx.enter_context(tc.tile_pool(name="m", bufs=2))
        pspool = ctx.enter_context(tc.tile_pool(name="ps", bufs=4, space="PSUM"))
        opool = ctx.enter_context(tc.tile_pool(name="o", bufs=3))

        tt = nc.vector.tensor_tensor
        for slot, (_u, off, wid) in enumerate(CHUNKS):
            nmm = wid // WB
            Z = zpool.tile([P, R, WMAX], bf16, tag="z", name=f"z{slot}")
            src = x_d[off:off + P * C * wid].rearrange(
                "(p c w) -> p c w", p=P, c=C, w=wid
            )
            nc.sync.dma_start(Z[:, 0:C, 0:wid], src)
            nc.gpsimd.memset(Z[:, 2 * C, 0:wid], 1.0)
            # squares on ScalarE (otherwise idle), into the moving tile
            nc.scalar.activation(
                Z[:, C:2 * C, 0:wid], Z[:, 0:C, 0:wid], Act.Square
            )

            # pairwise max tree over the 19 channel rows, all bf16 2x
            t = tpool.tile([P, 10, WMAX], bf16, tag="t", name=f"t{slot}")
            x = Z[:, 0:C, 0:wid]
            tv = t[:, :, 0:wid]
            tt(out=tv[:, 0:9], in0=x[:, 0:9], in1=x[:, 9:18], op=Alu.max)
            tt(out=tv[:, 0:4], in0=tv[:, 0:4], in1=tv[:, 4:8], op=Alu.max)
            tt(out=tv[:, 0:2], in0=tv[:, 0:2], in1=tv[:, 2:4], op=Alu.max)
            tt(out=tv[:, 0], in0=tv[:, 0], in1=tv[:, 1], op=Alu.max)
            tt(out=tv[:, 0], in0=tv[:, 0], in1=tv[:, 8], op=Alu.max)
            m = mpool.tile([P, WMAX], bf16, tag="m", name=f"m{slot}")
            mv = m[:, 0:wid]
            tt(out=mv, in0=tv[:, 0], in1=x[:, 18], op=Alu.max)

            # all 19 masks via broadcast compare (bf16 2x_1P), written in
            # w-blocked (g, c, wb) order so each group's stationary slice
            # coalesces to ONE free dim (the PE weights AP requires that);
            # split so the matmul stream starts before the far pieces finish
            eq = epool.tile([P, WMAX // WB, C, WB], bf16, tag="eq",
                            name=f"eq{slot}")
            x_blk = x.rearrange("p c (g b) -> p g c b", b=WB)
            m_blk = mv.rearrange("p (g b) -> p g b", b=WB)
            m_b = m_blk[:, :, None, :].broadcast_to([P, nmm, C, WB])
            nsplit = max(1, nmm // 16)
            qg = nmm // nsplit
            for q in range(nsplit):
                sl = slice(q * qg, (q + 1) * qg)
                tt(out=eq[:, sl], in0=x_blk[:, sl], in1=m_b[:, sl],
                   op=Alu.is_equal)

            # PSUM-accumulated masked sums: out[(c,wi),(r,wj)] +=
            #   sum_p eq[p,c,4g+wi] * Z[p,r,4g+wj]; host keeps wi==wj.
            # pad the PSUM tile to a full 2 KB bank so pool packing never
            # co-locates two accumulators in one bank (false serialization)
            ps_full = pspool.tile([SF, 512], f32, tag="ps", name=f"ps{slot}")
            ps = ps_full[:, 0:MF]
            for g in range(nmm):
                w0 = g * WB
                nc.tensor.matmul(
                    ps,
                    lhsT=eq[:, g],
                    rhs=Z[:, :, w0:w0 + WB],
                    start=(g == 0),
                    stop=(g == nmm - 1),
                )

            ob = opool.tile([SF, MF], f32, tag="o", name=f"o{slot}")
            nc.scalar.copy(out=ob[:], in_=ps)
            nc.sync.dma_start(out_d[slot], ob[:])

    nc.compile()
    return nc


def _get_nc():
    if "nc" not in _CACHE:
        _CACHE["nc"] = _build_nc()
    return _CACHE["nc"]


def _make_shards(logits):
    import ml_dtypes

    xb = np.ascontiguousarray(logits).astype(ml_dtypes.bfloat16)
    shards = []
    for k in range(NCORES):
        flat = np.empty(TOTELEMS, dtype=ml_dtypes.bfloat16)
        for (u, off, wid) in CHUNKS:
            n, s = UNITS[UPC * k + u]
            w0 = sum(w for (uu, _o, w) in CHUNKS if uu == u and _o < off)
            blk = xb[n, :, s * P:(s + 1) * P, w0:w0 + wid]   # (C, P, wid)
            flat[off:off + P * C * wid] = (
                blk.transpose(1, 0, 2).reshape(-1)           # (P, C, wid)
            )
        shards.append(flat)
    return shards


def _finish(results):
    per_n = np.zeros((N, 3, C), dtype=np.float64)
    for k in range(NCORES):
        arr = np.asarray(results[k]["stats"], dtype=np.float64)
        # (NSLOTS, SF=C*WB, MF=R*WB) -> diagonal over the WB w-phases
        a = arr.reshape(NSLOTS, C, WB, R, WB)
        g_all = np.einsum('scirj->scr', a * np.eye(WB)[None, None, :, None, :])
        for slot in range(NSLOTS):
            u, _off, _wid = CHUNKS[slot]
            n, _s = UNITS[UPC * k + u]
            g = g_all[slot]                          # (C, R)
            per_n[n, 0] += g[:, 2 * C]               # cnt
            per_n[n, 1] += g[:, 0:C].sum(axis=1)     # S1
            per_n[n, 2] += g[:, C:2 * C].sum(axis=1)  # S2
    cnt, S1, S2 = per_n[:, 0], per_n[:, 1], per_n[:, 2]
    K = np.maximum(cnt, 1.0) * C
    sq_dev = np.maximum(S2 - S1 * S1 / K, 0.0)
    norms = np.where(cnt > 0, np.sqrt(sq_dev), 0.0)
    loss = norms.mean(axis=0).sum()
    return np.array(loss, dtype=np.float32)


def kernel(**inputs):
    _ensure_ntff_hook()
    from concourse.bass_utils import run_bass_kernel_spmd

    logits = np.asarray(inputs["logits"])
    assert logits.shape == (N, C, H, W), logits.shape
    nc = _get_nc()
    shards = _make_shards(np.asarray(logits, dtype=np.float32))
    in_maps = [{"x": shards[k]} for k in range(NCORES)]
    res = run_bass_kernel_spmd(nc, in_maps, list(range(NCORES)))
    return _finish(res.results)
